# revision 1
# baseline (speedup 1.0000x reference)
"""GNN NodeBlock kernel for 8 Trainium2 NeuronCores.

Strategy: shard edges by DESTINATION node (core c owns nodes
[c*12500, (c+1)*12500) and every edge pointing at them). Each core then
computes its node slice end-to-end; the only cross-core traffic is two
tiny BatchNorm-statistics all-reduces (2x64 floats each).

Pipeline per core:
  pass1: gather x[row] (indirect DMA) -> PE transpose -> concat with
         pre-transposed edge_attr -> W1a matmul -> h1^T (feature-major),
         stored bf16 in SBUF, BN1 sums/sumsq via fused ACT accum.
  AR1:   all-reduce BN1 stats, fold into per-feature scale/bias.
  pass2: fused scale+bias+ReLU (ACT) -> PE transpose to edge-major ->
         one-hot (iota is_equal) scatter-matmul into per-128-node-block
         PSUM accumulators [128, 65] (64 feature sums + edge count).
  node:  mean = sums * recip(max(cnt,1)); indicator row folds b2a into
         an extended W2a matmul; concat with x^T slice; W1b; BN2 stats
         all-reduce; ReLU; W2b + b2b; output feature-major [64, nodes].
Host stitches per-core outputs (transpose + concat).

Linear biases feeding BatchNorm (b1a, b1b) cancel exactly and are
dropped. Padding edges gather a zero x-row and carry col_rel=-1 so
they contribute nothing to stats, sums, or counts.
"""

import sys

for _p in ("/opt/trn_rl_repo", "/opt/pypackages"):
    if _p not in sys.path:
        sys.path.insert(0, _p)

import numpy as np

N = 100000
E = 800000
F = 64          # feature width (INPUTS == HIDDEN == 64)
DIN = 128       # MLP input dim
EPS = 1e-5
NCORES = 8
NPC = N // NCORES          # 12500 real nodes per core
NB = (NPC + 127) // 128    # 98 node blocks per core
NN = NB * 128              # 12544 padded nodes per core
P = 128

_BUILD_CACHE = {}


# --------------------------------------------------------------------------
# Host-side sharding
# --------------------------------------------------------------------------

def _shard_inputs(x, edge_index, edge_attr):
    """Bucket edges by destination core, sort by destination node, pad each
    128-node block's edge list to C chunks of 128 edges (uniform C across
    cores/blocks so the SPMD program is static). Ships the per-core edge
    MLP input pre-assembled feature-major in bf16: zt[0:64] = x[row].T,
    zt[64:128] = edge_attr.T."""
    import ml_dtypes
    bf16 = ml_dtypes.bfloat16

    row = np.asarray(edge_index[0], dtype=np.int64)
    col = np.asarray(edge_index[1], dtype=np.int64)
    x = np.ascontiguousarray(np.asarray(x, dtype=np.float32))
    edge_attr = np.ascontiguousarray(np.asarray(edge_attr, dtype=np.float32))

    owner = col // NPC
    per_core = []
    max_blk_cnt = 0
    for c in range(NCORES):
        sel = np.flatnonzero(owner == c)
        rel = col[sel] - c * NPC
        order = np.argsort(rel, kind="stable")
        sel = sel[order]
        rel = rel[order]
        blk = rel >> 7
        cnts = np.bincount(blk, minlength=NB)
        max_blk_cnt = max(max_blk_cnt, int(cnts.max()))
        per_core.append((sel, rel, cnts))

    C = max(1, -(-max_blk_cnt // 128))        # chunks per block
    n_chunks = NB * C
    n_chunks += (-n_chunks) % 4               # groups of 4 chunks (512 edges)
    E_pad = n_chunks * 128

    xT16 = x.T.astype(bf16)                   # [64, N]
    in_maps = []
    for c in range(NCORES):
        sel, rel, cnts = per_core[c]
        col_rel = np.full(E_pad, -1.0, dtype=np.float32)
        zt = np.zeros((DIN, E_pad), dtype=bf16)

        starts = np.zeros(NB, dtype=np.int64)
        starts[1:] = np.cumsum(cnts)[:-1]
        blk = (rel >> 7).astype(np.int64)
        slot = blk * (C * 128) + (np.arange(len(sel)) - starts[blk])

        col_rel[slot] = (rel & 127).astype(np.float32)
        zt[0:F, slot] = xT16[:, row[sel]]
        zt[F:DIN, slot] = edge_attr[sel].astype(bf16).T

        xt_slice = np.zeros((F, NN), dtype=np.float32)
        xt_slice[:, :NPC] = x[c * NPC:(c + 1) * NPC].T

        in_maps.append({
            "zt": zt,
            "colrel": np.ascontiguousarray(col_rel.reshape(n_chunks, 128).T),
            "xt_slice": xt_slice,
        })
    return in_maps, C, n_chunks


# --------------------------------------------------------------------------
# Device program
# --------------------------------------------------------------------------

def _build_program(C, n_chunks):
    from concourse import bass, mybir, tile, bacc
    from concourse.masks import make_identity

    f32 = mybir.dt.float32
    bf16 = mybir.dt.bfloat16
    i32 = mybir.dt.int32
    AF = mybir.ActivationFunctionType
    OP = mybir.AluOpType

    E_pad = n_chunks * 128
    n_groups = n_chunks // 4          # 512 edges per group
    G1 = -(-n_groups // 2)            # groups stored on partitions 0..63
    HLEN = G1 * 512
    NGRP = NB * C                     # chunks that carry real blocks
    rg = [list(range(NCORES))]

    nc = bacc.Bacc("TRN2", target_bir_lowering=False, debug=False,
                   enable_asserts=False, num_devices=NCORES)

    def inp(name, shape, dt=f32):
        return nc.dram_tensor(name, list(shape), dt, kind="ExternalInput")

    zt_d = inp("zt", (DIN, E_pad), bf16)
    colrel = inp("colrel", (P, n_chunks))
    xt_slice = inp("xt_slice", (F, NN))
    w1a_d = inp("w1a", (DIN, F), bf16)
    w2a_d = inp("w2a_ext", (F + 1, F))
    w1b_d = inp("w1b", (DIN, F))
    w2b_d = inp("w2b", (F, F))
    bn1_d = inp("bn1", (F, 2))        # col0 g1, col1 be1
    bn2_d = inp("bn2", (F, 2))
    b2b_d = inp("b2b_col", (F, 1))
    out_d = nc.dram_tensor("outT", [F, NN], f32, kind="ExternalOutput")

    def h1_slice(g, lo=0, hi=512):
        if g < G1:
            return h1_store[0:64, g * 512 + lo: g * 512 + hi]
        g -= G1
        return h1_store[64:128, g * 512 + lo: g * 512 + hi]

    with tile.TileContext(nc) as tc:
        with (
            tc.tile_pool(name="persist", bufs=1) as pp,
            tc.tile_pool(name="dram", bufs=1, space="DRAM") as dp,
        ):
            ident = pp.tile([P, P], bf16)
            make_identity(nc, ident[:])
            ident32 = pp.tile([P, P], f32)
            make_identity(nc, ident32[:])
            iota_t = pp.tile([P, P], f32)
            nc.gpsimd.iota(iota_t[:], pattern=[[1, P]], base=0,
                           channel_multiplier=0,
                           allow_small_or_imprecise_dtypes=True)

            w1a = pp.tile([DIN, F], bf16)
            nc.sync.dma_start(out=w1a[:], in_=w1a_d[:])
            w2a = pp.tile([F + 1, F], f32)
            nc.sync.dma_start(out=w2a[:], in_=w2a_d[:])
            w1b = pp.tile([DIN, F], f32)
            nc.sync.dma_start(out=w1b[:], in_=w1b_d[:])
            w2b = pp.tile([F, F], f32)
            nc.sync.dma_start(out=w2b[:], in_=w2b_d[:])
            bn1 = pp.tile([F, 2], f32)
            nc.sync.dma_start(out=bn1[:], in_=bn1_d[:])
            bn2 = pp.tile([F, 2], f32)
            nc.sync.dma_start(out=bn2[:], in_=bn2_d[:])
            b2b = pp.tile([F, 1], f32)
            nc.sync.dma_start(out=b2b[:], in_=b2b_d[:])

            sums_sb = pp.tile([P, NB * 65], f32)    # per-block node sums+cnt
            s1 = pp.tile([F, 1], f32)
            t1 = pp.tile([F, 1], f32)
            s2 = pp.tile([F, 1], f32)
            t2 = pp.tile([F, 1], f32)

            def bn_fold(stats_all, bn_w, s_out, t_out, inv_n, tag):
                """s = g*rsqrt(var+eps); t = be - mu*s from summed stats."""
                mu = pp.tile([F, 1], f32, tag=f"mu{tag}")
                msq = pp.tile([F, 1], f32, tag=f"msq{tag}")
                nc.vector.tensor_scalar_mul(out=mu[:], in0=stats_all[:, 0:1],
                                            scalar1=inv_n)
                nc.vector.tensor_scalar_mul(out=msq[:], in0=stats_all[:, 1:2],
                                            scalar1=inv_n)
                var = pp.tile([F, 1], f32, tag=f"var{tag}")
                nc.vector.tensor_tensor(out=var[:], in0=mu[:], in1=mu[:],
                                        op=OP.mult)
                nc.vector.tensor_tensor(out=var[:], in0=msq[:], in1=var[:],
                                        op=OP.subtract)
                nc.vector.tensor_scalar_add(out=var[:], in0=var[:],
                                            scalar1=float(EPS))
                sd = pp.tile([F, 1], f32, tag=f"sd{tag}")
                nc.scalar.activation(out=sd[:], in_=var[:], func=AF.Sqrt,
                                     bias=0.0, scale=1.0)
                rsd = pp.tile([F, 1], f32, tag=f"rsd{tag}")
                nc.vector.reciprocal(out=rsd[:], in_=sd[:])
                nc.vector.tensor_tensor(out=s_out[:], in0=rsd[:],
                                        in1=bn_w[:, 0:1], op=OP.mult)
                nc.vector.tensor_tensor(out=t_out[:], in0=mu[:], in1=s_out[:],
                                        op=OP.mult)
                nc.vector.tensor_tensor(out=t_out[:], in0=bn_w[:, 1:2],
                                        in1=t_out[:], op=OP.subtract)

            # ---------------- edge phase (pass1 + AR1 + pass2) ------------
            with (
                tc.tile_pool(name="edge_persist", bufs=1) as ep,
                tc.tile_pool(name="work", bufs=3) as wp,
                tc.tile_pool(name="psum1", bufs=2, space="PSUM") as psp,
            ):
                h1_store = ep.tile([P, HLEN], bf16)
                colrel_sb = ep.tile([P, n_chunks], f32)
                nc.sync.dma_start(out=colrel_sb[:], in_=colrel[:])
                sum1p = ep.tile([F, n_groups], f32)
                sq1p = ep.tile([F, n_groups], f32)

                # ---- pass 1 ----
                for g in range(n_groups):
                    zT = wp.tile([P, 512], bf16, tag="zT")
                    nc.sync.dma_start(out=zT[:],
                                      in_=zt_d[:, g * 512:(g + 1) * 512])
                    ps_h1 = psp.tile([F, 512], f32, tag="ps_h1", space="PSUM")
                    nc.tensor.matmul(out=ps_h1[:], lhsT=w1a[:], rhs=zT[:],
                                     start=True, stop=True)
                    nc.scalar.activation(out=h1_slice(g), in_=ps_h1[:],
                                         func=AF.Copy,
                                         accum_out=sum1p[:, g:g + 1])
                    sq = wp.tile([F, 512], f32, tag="sq")
                    nc.scalar.activation(out=sq[:], in_=ps_h1[:],
                                         func=AF.Square,
                                         accum_out=sq1p[:, g:g + 1])

                # ---- BN1 stats all-reduce ----
                st1 = ep.tile([F, 2], f32)
                nc.vector.tensor_reduce(out=st1[:, 0:1], in_=sum1p[:],
                                        axis=mybir.AxisListType.X, op=OP.add)
                nc.vector.tensor_reduce(out=st1[:, 1:2], in_=sq1p[:],
                                        axis=mybir.AxisListType.X, op=OP.add)
                cc1_in = dp.tile([F, 2], f32)
                cc1_out = dp.tile([F, 2], f32)
                nc.gpsimd.dma_start(out=cc1_in[:], in_=st1[:])
                nc.gpsimd.collective_compute(
                    "AllReduce", OP.add, replica_groups=rg,
                    ins=[cc1_in.opt()], outs=[cc1_out.opt()])
                st1a = ep.tile([F, 2], f32)
                nc.gpsimd.dma_start(out=st1a[:], in_=cc1_out[:])
                bn_fold(st1a, bn1, s1, t1, 1.0 / E, "1")

                # ---- pass 2: scatter ----
                r_tiles = []
                for k in range(3):
                    rt = ep.tile([P, 65], bf16, tag=f"r{k}")
                    nc.gpsimd.memset(rt[:, 64:65], 1.0)
                    r_tiles.append(rt)

                ps_blk = None
                for g in range(n_groups):
                    rT = wp.tile([F, 512], bf16, tag="rT")
                    nc.scalar.activation(out=rT[:], in_=h1_slice(g),
                                         func=AF.Relu, bias=t1[:, 0:1],
                                         scale=s1[:, 0:1])
                    for j in range(4):
                        ch = 4 * g + j
                        if ch >= NGRP:
                            continue        # tail pad chunks: no block
                        b = ch // C
                        first = (ch % C == 0)
                        last = (ch % C == C - 1)
                        ps_r = psp.tile([P, F], bf16, tag="ps_r", space="PSUM")
                        nc.tensor.transpose(out=ps_r[:],
                                            in_=rT[:, j * 128:(j + 1) * 128],
                                            identity=ident[0:64, 0:64])
                        rt = r_tiles[ch % 3]
                        nc.vector.tensor_copy(out=rt[:, 0:64], in_=ps_r[:])
                        oh = wp.tile([P, P], bf16, tag="oh")
                        nc.vector.tensor_tensor(
                            out=oh[:],
                            in0=colrel_sb[:, ch:ch + 1].to_broadcast([P, P]),
                            in1=iota_t[:], op=OP.is_equal)
                        if first:
                            ps_blk = psp.tile([P, 65], f32, tag="ps_blk",
                                              space="PSUM")
                        nc.tensor.matmul(out=ps_blk[:], lhsT=oh[:],
                                         rhs=rt[:, 0:65],
                                         start=first, stop=last)
                        if last:
                            nc.vector.tensor_copy(
                                out=sums_sb[:, b * 65:(b + 1) * 65],
                                in_=ps_blk[:])

            # ---------------- node phase ---------------------------------
            with (
                tc.tile_pool(name="node_persist", bufs=1) as np_,
                tc.tile_pool(name="nwork", bufs=3) as nw,
                tc.tile_pool(name="psum2", bufs=2, space="PSUM") as ps2,
            ):
                h2_store = np_.tile([F, NN], f32)
                n_ngrp = -(-NN // 512)
                sum2p = np_.tile([F, n_ngrp], f32)
                sq2p = np_.tile([F, n_ngrp], f32)

                widths = []
                off = 0
                while off < NN:
                    w = min(512, NN - off)
                    widths.append((off, w))
                    off += w

                for ng, (off, w) in enumerate(widths):
                    nblk = w // 128
                    z2 = nw.tile([P, w], f32, tag="z2")
                    nc.sync.dma_start(out=z2[0:64, :],
                                      in_=xt_slice[:, off:off + w])
                    mT = nw.tile([F + 1, w], f32, tag="mT")
                    for j in range(nblk):
                        b = off // 128 + j
                        cnt = sums_sb[:, b * 65 + 64:b * 65 + 65]
                        cntc = nw.tile([P, 1], f32, tag="cntc")
                        nc.vector.tensor_scalar_max(out=cntc[:], in0=cnt,
                                                    scalar1=1.0)
                        rec = nw.tile([P, 1], f32, tag="rec")
                        nc.vector.reciprocal(out=rec[:], in_=cntc[:])
                        me = nw.tile([P, 65], f32, tag="me")
                        nc.vector.tensor_tensor(
                            out=me[:, 0:64],
                            in0=sums_sb[:, b * 65:b * 65 + 64],
                            in1=rec[:].to_broadcast([P, F]), op=OP.mult)
                        nc.vector.tensor_scalar_min(out=me[:, 64:65], in0=cnt,
                                                    scalar1=1.0)
                        ps_mT = ps2.tile([F + 1, P], f32, tag="ps_mT",
                                         space="PSUM")
                        nc.tensor.transpose(out=ps_mT[:], in_=me[:],
                                            identity=ident32[:])
                        nc.vector.tensor_copy(
                            out=mT[:, j * 128:(j + 1) * 128], in_=ps_mT[:])
                    ps_msg = ps2.tile([F, w], f32, tag="ps_msg", space="PSUM")
                    nc.tensor.matmul(out=ps_msg[:], lhsT=w2a[:], rhs=mT[:],
                                     start=True, stop=True)
                    nc.scalar.activation(out=z2[64:128, :], in_=ps_msg[:],
                                         func=AF.Copy)
                    ps_h2 = ps2.tile([F, w], f32, tag="ps_h2", space="PSUM")
                    nc.tensor.matmul(out=ps_h2[:], lhsT=w1b[:], rhs=z2[:],
                                     start=True, stop=True)
                    nc.scalar.activation(out=h2_store[:, off:off + w],
                                         in_=ps_h2[:], func=AF.Copy,
                                         accum_out=sum2p[:, ng:ng + 1])
                    sq2 = nw.tile([F, w], f32, tag="sq2")
                    nc.scalar.activation(out=sq2[:], in_=ps_h2[:],
                                         func=AF.Square,
                                         accum_out=sq2p[:, ng:ng + 1])

                # ---- BN2 stats all-reduce ----
                st2 = np_.tile([F, 2], f32)
                nc.vector.tensor_reduce(out=st2[:, 0:1], in_=sum2p[:],
                                        axis=mybir.AxisListType.X, op=OP.add)
                nc.vector.tensor_reduce(out=st2[:, 1:2], in_=sq2p[:],
                                        axis=mybir.AxisListType.X, op=OP.add)
                cc2_in = dp.tile([F, 2], f32)
                cc2_out = dp.tile([F, 2], f32)
                nc.gpsimd.dma_start(out=cc2_in[:], in_=st2[:])
                nc.gpsimd.collective_compute(
                    "AllReduce", OP.add, replica_groups=rg,
                    ins=[cc2_in.opt()], outs=[cc2_out.opt()])
                st2a = np_.tile([F, 2], f32)
                nc.gpsimd.dma_start(out=st2a[:], in_=cc2_out[:])
                bn_fold(st2a, bn2, s2, t2, 1.0 / N, "2")

                # ---- output ----
                for ng, (off, w) in enumerate(widths):
                    rT2 = nw.tile([F, w], f32, tag="rT2")
                    nc.scalar.activation(out=rT2[:],
                                         in_=h2_store[:, off:off + w],
                                         func=AF.Relu, bias=t2[:, 0:1],
                                         scale=s2[:, 0:1])
                    ps_o = ps2.tile([F, w], f32, tag="ps_o", space="PSUM")
                    nc.tensor.matmul(out=ps_o[:], lhsT=w2b[:], rhs=rT2[:],
                                     start=True, stop=True)
                    oT = nw.tile([F, w], f32, tag="oT")
                    nc.scalar.activation(out=oT[:], in_=ps_o[:],
                                         func=AF.Identity, bias=b2b[:, 0:1],
                                         scale=1.0)
                    nc.sync.dma_start(out=out_d[:, off:off + w], in_=oT[:])

    nc.compile()
    return nc


# --------------------------------------------------------------------------
# Entry point
# --------------------------------------------------------------------------

def kernel(x, edge_index, edge_attr, u, batch,
           W1a, b1a, g1, be1, W2a, b2a,
           W1b, b1b, g2, be2, W2b, b2b, **_unused):
    from concourse.bass_utils import run_bass_kernel_spmd

    in_maps, C, n_chunks = _shard_inputs(x, edge_index, edge_attr)

    key = (C, n_chunks)
    if key not in _BUILD_CACHE:
        _BUILD_CACHE[key] = _build_program(C, n_chunks)
    nc = _BUILD_CACHE[key]

    w2a_ext = np.concatenate(
        [np.asarray(W2a, np.float32),
         np.asarray(b2a, np.float32)[None, :]], axis=0)
    bn1 = np.stack([np.asarray(g1, np.float32),
                    np.asarray(be1, np.float32)], axis=1)
    bn2 = np.stack([np.asarray(g2, np.float32),
                    np.asarray(be2, np.float32)], axis=1)
    import ml_dtypes
    weights = {
        "w1a": np.ascontiguousarray(
            np.asarray(W1a, np.float32).astype(ml_dtypes.bfloat16)),
        "w2a_ext": np.ascontiguousarray(w2a_ext),
        "w1b": np.ascontiguousarray(np.asarray(W1b, np.float32)),
        "w2b": np.ascontiguousarray(np.asarray(W2b, np.float32)),
        "bn1": np.ascontiguousarray(bn1),
        "bn2": np.ascontiguousarray(bn2),
        "b2b_col": np.ascontiguousarray(
            np.asarray(b2b, np.float32)[:, None]),
    }
    for m in in_maps:
        m.update(weights)

    res = run_bass_kernel_spmd(nc, in_maps, core_ids=list(range(NCORES)))

    out = np.empty((N, F), dtype=np.float32)
    for c in range(NCORES):
        out[c * NPC:(c + 1) * NPC] = res.results[c]["outT"].T[:NPC]
    return out



# revision 9
# speedup vs baseline: 1.8259x; 1.8259x over previous
"""GNN NodeBlock kernel for 8 Trainium2 NeuronCores.

Strategy: shard edges by DESTINATION node. Host bin-packs each core's
12500 nodes into NSUP blocks of <=128 nodes and <=1024 edges (snake
deal by degree), so every block is exactly one 1024-edge "super chunk"
and the SPMD program is static and uniform. Each core computes its
node slice end-to-end; the only cross-core traffic is two tiny
BatchNorm-statistics all-reduces (2x64 floats each).

Pipeline per core (everything packed onto all 128 partitions):
  pass1: DMA zt super [128,1024] bf16 -> two W1a matmuls (zero-padded
         lhsT halves) accumulate into ONE [128,512] PSUM (A-edges on
         partitions 0:63, B-edges on 64:127) -> one ACT copy to bf16
         h1_store + one DVE bn_stats for BN1 statistics.
  AR1:   merge bn stats (bn_aggr + partition-half fold) -> all-reduce
         sums/sumsq -> fold into per-feature scale/bias (dup to 128).
  pass2: one packed ReLU ACT per super -> 4 PE transposes into one
         PSUM bank -> one bf16 DVE copy -> 8 one-hot columns built via
         tensor_scalar(is_equal) against a bf16 iota (4x DVE mode) ->
         8 scatter matmuls accumulate the block's node sums.
  node:  mean = sums * recip (host-precomputed 1/max(cnt,1)); mT gets
         an indicator row (host min(cnt,1)) so b2a folds into W2a;
         pairs of 512-node groups pack h2 [128,512]; BN2 stats via
         bn_stats; output pass uses a block-diagonal W2b matmul.
Host stitches per-core outputs (unpack + inverse node permutation).

Linear biases feeding BatchNorm (b1a, b1b) cancel exactly and are
dropped. Padding edges carry zt=0 and colrel=-1 so they contribute
nothing to stats, sums, or counts.
"""

import sys

for _p in ("/opt/trn_rl_repo", "/opt/pypackages"):
    if _p not in sys.path:
        sys.path.insert(0, _p)

import numpy as np

N = 100000
E = 800000
F = 64          # feature width (INPUTS == HIDDEN == 64)
DIN = 128       # MLP input dim
EPS = 1e-5
NCORES = 8
NPC = N // NCORES          # 12500 real nodes per core
P = 128
BLK_EDGES = 1024           # edges per block (8 chunks of 128)
BLK_NODES = 128

_BUILD_CACHE = {}


# --------------------------------------------------------------------------
# Host-side sharding
# --------------------------------------------------------------------------

def _pack_bins(deg):
    """Snake-deal nodes (sorted by degree desc) into bins with <=BLK_NODES
    nodes and <=BLK_EDGES edges each. Returns (nbins, bin_of, pos_of)."""
    n = deg.shape[0]
    order = np.argsort(-deg, kind="stable")
    etot = int(deg.sum())
    nbins = max(int(np.ceil(etot / (BLK_EDGES - 24))),
                int(np.ceil(n / (BLK_NODES - 1))))
    while True:
        nrows = -(-n // nbins)
        padded = np.full(nrows * nbins, -1, dtype=np.int64)
        padded[:n] = order
        grid = padded.reshape(nrows, nbins)
        grid[1::2] = grid[1::2, ::-1]          # snake
        bin_of = np.empty(n, dtype=np.int64)
        pos_of = np.empty(n, dtype=np.int64)
        colidx = np.tile(np.arange(nbins), (nrows, 1))
        rowidx = np.tile(np.arange(nrows)[:, None], (1, nbins))
        valid = grid >= 0
        bin_of[grid[valid]] = colidx[valid]
        pos_of[grid[valid]] = rowidx[valid]
        esum = np.bincount(bin_of, weights=deg.astype(np.float64),
                           minlength=nbins)
        ncnt = np.bincount(bin_of, minlength=nbins)
        if esum.max() <= BLK_EDGES and ncnt.max() <= BLK_NODES:
            return nbins, bin_of, pos_of
        nbins += 1


def _shard_inputs(x, edge_index, edge_attr):
    import ml_dtypes
    bf16 = ml_dtypes.bfloat16

    row = np.asarray(edge_index[0], dtype=np.int64)
    col = np.asarray(edge_index[1], dtype=np.int64)
    x = np.ascontiguousarray(np.asarray(x, dtype=np.float32))
    edge_attr = np.ascontiguousarray(np.asarray(edge_attr, dtype=np.float32))
    xT16 = x.T.astype(bf16)                    # [64, N]

    owner = col // NPC
    packs = []
    nsup = 0
    for c in range(NCORES):
        sel = np.flatnonzero(owner == c)
        rel = col[sel] - c * NPC
        deg = np.bincount(rel, minlength=NPC)
        nbins, bin_of, pos_of = _pack_bins(deg)
        packs.append((sel, rel, bin_of, pos_of, deg, nbins))
        nsup = max(nsup, nbins)
    nsup = -(-nsup // 8) * 8                   # NG even, pairs align
    e_pad = nsup * BLK_EDGES
    nslots = nsup * BLK_NODES

    in_maps = []
    perms = []
    for c in range(NCORES):
        sel, rel, bin_of, pos_of, deg, nbins = packs[c]

        # node slot <- original local node
        perm = np.full(nslots, -1, dtype=np.int64)
        perm[bin_of * BLK_NODES + pos_of] = np.arange(NPC)
        perms.append(perm)

        # edge slots: edges grouped by destination bin, sequential inside
        ebin = bin_of[rel]
        eorder = np.argsort(ebin, kind="stable")
        starts = np.zeros(nsup, dtype=np.int64)
        cnts_bin = np.bincount(ebin, minlength=nsup)
        starts[1:] = np.cumsum(cnts_bin)[:-1]
        slot = ebin[eorder] * BLK_EDGES + (np.arange(len(sel)) - starts[ebin[eorder]])
        esel = sel[eorder]

        colrel = np.full(e_pad, -1.0, dtype=np.float32)
        colrel[slot] = pos_of[rel[eorder]].astype(np.float32)
        zt = np.zeros((DIN, e_pad), dtype=bf16)
        zt[0:F, slot] = xT16[:, row[esel]]
        zt[F:DIN, slot] = edge_attr[esel].astype(bf16).T

        cnt_slot = np.zeros(nslots, dtype=np.float32)
        valid = perm >= 0
        cnt_slot[valid] = deg[perm[valid]]
        recip = 1.0 / np.maximum(cnt_slot, 1.0)
        ind = np.minimum(cnt_slot, 1.0)

        xt_slice = np.zeros((F, nslots), dtype=bf16)
        xt_slice[:, valid] = xT16[:, c * NPC + perm[valid]]

        in_maps.append({
            "zt": np.ascontiguousarray(zt),
            "colrel": np.ascontiguousarray(
                colrel.reshape(nsup * 8, 128).T),
            "recip": np.ascontiguousarray(recip.reshape(nsup, 128).T),
            "ind": np.ascontiguousarray(
                ind.reshape(nsup, 128).T.astype(bf16)),
            "xt_slice": xt_slice,
        })
    return in_maps, perms, nsup


# --------------------------------------------------------------------------
# Device program
# --------------------------------------------------------------------------

def _build_program(NSUP):
    from concourse import bass, mybir, tile, bacc
    from concourse.masks import make_identity

    f32 = mybir.dt.float32
    f32r = mybir.dt.float32r
    bf16 = mybir.dt.bfloat16
    AF = mybir.ActivationFunctionType
    OP = mybir.AluOpType

    E_pad = NSUP * BLK_EDGES
    NG = NSUP // 4                  # 512-node groups
    NPAIR = NG // 2
    rg = [list(range(NCORES))]

    nc = bacc.Bacc("TRN2", target_bir_lowering=False, debug=False,
                   enable_asserts=False, num_devices=NCORES)

    def inp(name, shape, dt=f32):
        return nc.dram_tensor(name, list(shape), dt, kind="ExternalInput")

    zt_d = inp("zt", (DIN, E_pad), bf16)
    colrel_d = inp("colrel", (P, NSUP * 8))
    recip_d = inp("recip", (P, NSUP))
    ind_d = inp("ind", (P, NSUP), bf16)
    xt_d = inp("xt_slice", (F, NSUP * P), bf16)
    w1a2_d = inp("w1a2", (DIN, 2 * P), bf16)     # [w1a|0] , [0|w1a]
    w2a_d = inp("w2a_ext", (F + 1, F), bf16)
    w1b2_d = inp("w1b2", (DIN, 2 * P), bf16)
    w2bd_d = inp("w2b_bd", (DIN, P), bf16)       # block-diag [[w2b,0],[0,w2b]]
    bn1_d = inp("bn1", (F, 2))                   # col0 g1, col1 be1
    bn2_d = inp("bn2", (F, 2))
    b2b_d = inp("b2b_dup", (P, 1))
    out_d = nc.dram_tensor("outT", [P, NPAIR * 512], f32,
                           kind="ExternalOutput")

    with tile.TileContext(nc) as tc:
        with (
            tc.tile_pool(name="persist", bufs=1) as pp,
            tc.tile_pool(name="dram", bufs=1, space="DRAM") as dp,
        ):
            ident = pp.tile([P, P], bf16)
            make_identity(nc, ident[:])
            iota_f = pp.tile([P, P], f32)
            nc.gpsimd.iota(iota_f[:], pattern=[[1, P]], base=0,
                           channel_multiplier=0,
                           allow_small_or_imprecise_dtypes=True)
            iota_bf = pp.tile([P, P], bf16)
            nc.vector.tensor_copy(out=iota_bf[:], in_=iota_f[:])

            w1a2 = pp.tile([DIN, 2 * P], bf16)
            nc.sync.dma_start(out=w1a2[:], in_=w1a2_d[:])
            w2a = pp.tile([F + 1, F], bf16)
            nc.sync.dma_start(out=w2a[:], in_=w2a_d[:])
            w1b2 = pp.tile([DIN, 2 * P], bf16)
            nc.sync.dma_start(out=w1b2[:], in_=w1b2_d[:])
            w2bd = pp.tile([DIN, P], bf16)
            nc.sync.dma_start(out=w2bd[:], in_=w2bd_d[:])
            bn1 = pp.tile([F, 2], f32)
            nc.sync.dma_start(out=bn1[:], in_=bn1_d[:])
            bn2 = pp.tile([F, 2], f32)
            nc.sync.dma_start(out=bn2[:], in_=bn2_d[:])
            b2b = pp.tile([P, 1], f32)
            nc.sync.dma_start(out=b2b[:], in_=b2b_d[:])
            recip_sb = pp.tile([P, NSUP], f32)
            nc.sync.dma_start(out=recip_sb[:], in_=recip_d[:])
            ind_sb = pp.tile([P, NSUP], bf16)
            nc.sync.dma_start(out=ind_sb[:], in_=ind_d[:])

            sums_sb = pp.tile([P, NSUP * F], f32)   # per-block node sums
            s1d = pp.tile([P, 1], f32)
            t1d = pp.tile([P, 1], f32)
            s2d = pp.tile([P, 1], f32)
            t2d = pp.tile([P, 1], f32)

            def fold_stats(bnst, nsub, n_half, bn_w, inv_n, s_out, t_out, tag):
                """bn_stats tiles -> merged (sum, sumsq) -> AllReduce ->
                s = g*rsqrt(var+eps), t = be - mu*s, duplicated to 128."""
                ag = pp.tile([P, 2], f32, tag=f"ag{tag}")
                nc.vector.bn_aggr(out=ag[:], in_=bnst[:])
                # (mean, var) -> (sum, sumsq) per partition
                ss = pp.tile([P, 2], f32, tag=f"ss{tag}")
                nc.vector.tensor_scalar_mul(out=ss[:, 0:1], in0=ag[:, 0:1],
                                            scalar1=float(n_half))
                msq = pp.tile([P, 1], f32, tag=f"msq{tag}")
                nc.vector.tensor_tensor(out=msq[:], in0=ag[:, 0:1],
                                        in1=ag[:, 0:1], op=OP.mult)
                nc.vector.tensor_tensor(out=msq[:], in0=ag[:, 1:2],
                                        in1=msq[:], op=OP.add)
                nc.vector.tensor_scalar_mul(out=ss[:, 1:2], in0=msq[:],
                                            scalar1=float(n_half))
                # fold partition halves: feature f = p[f] + p[f+64]
                hi = pp.tile([F, 2], f32, tag=f"hi{tag}")
                nc.sync.dma_start(out=hi[:], in_=ss[F:P, :])
                st = pp.tile([F, 2], f32, tag=f"st{tag}")
                nc.vector.tensor_tensor(out=st[:], in0=ss[0:F, :],
                                        in1=hi[:], op=OP.add)
                cc_in = dp.tile([F, 2], f32, tag=f"cci{tag}")
                cc_out = dp.tile([F, 2], f32, tag=f"cco{tag}")
                nc.gpsimd.dma_start(out=cc_in[:], in_=st[:])
                nc.gpsimd.collective_compute(
                    "AllReduce", OP.add, replica_groups=rg,
                    ins=[cc_in.opt()], outs=[cc_out.opt()])
                sta = pp.tile([F, 2], f32, tag=f"sta{tag}")
                nc.gpsimd.dma_start(out=sta[:], in_=cc_out[:])
                mu = pp.tile([F, 1], f32, tag=f"mu{tag}")
                msq2 = pp.tile([F, 1], f32, tag=f"msq2{tag}")
                nc.vector.tensor_scalar_mul(out=mu[:], in0=sta[:, 0:1],
                                            scalar1=inv_n)
                nc.vector.tensor_scalar_mul(out=msq2[:], in0=sta[:, 1:2],
                                            scalar1=inv_n)
                var = pp.tile([F, 1], f32, tag=f"var{tag}")
                nc.vector.tensor_tensor(out=var[:], in0=mu[:], in1=mu[:],
                                        op=OP.mult)
                nc.vector.tensor_tensor(out=var[:], in0=msq2[:], in1=var[:],
                                        op=OP.subtract)
                nc.vector.tensor_scalar_add(out=var[:], in0=var[:],
                                            scalar1=float(EPS))
                sd = pp.tile([F, 1], f32, tag=f"sd{tag}")
                nc.scalar.activation(out=sd[:], in_=var[:], func=AF.Sqrt,
                                     bias=0.0, scale=1.0)
                rsd = pp.tile([F, 1], f32, tag=f"rsd{tag}")
                nc.vector.reciprocal(out=rsd[:], in_=sd[:])
                s_lo = pp.tile([F, 1], f32, tag=f"slo{tag}")
                t_lo = pp.tile([F, 1], f32, tag=f"tlo{tag}")
                nc.vector.tensor_tensor(out=s_lo[:], in0=rsd[:],
                                        in1=bn_w[:, 0:1], op=OP.mult)
                nc.vector.tensor_tensor(out=t_lo[:], in0=mu[:], in1=s_lo[:],
                                        op=OP.mult)
                nc.vector.tensor_tensor(out=t_lo[:], in0=bn_w[:, 1:2],
                                        in1=t_lo[:], op=OP.subtract)
                nc.vector.tensor_copy(out=s_out[0:F, :], in_=s_lo[:])
                nc.sync.dma_start(out=s_out[F:P, :], in_=s_lo[:])
                nc.vector.tensor_copy(out=t_out[0:F, :], in_=t_lo[:])
                nc.sync.dma_start(out=t_out[F:P, :], in_=t_lo[:])

            # ---------------- edge phase (pass1 + AR1 + pass2) ------------
            with (
                tc.tile_pool(name="edge_persist", bufs=1) as ep,
                tc.tile_pool(name="work", bufs=3) as wp,
                tc.tile_pool(name="psum1", bufs=3, space="PSUM") as psp,
            ):
                h1_store = ep.tile([P, NSUP * 512], bf16)
                bnst1 = ep.tile([P, NSUP * 6], f32)
                colrel_sb = ep.tile([P, NSUP * 8], f32)
                nc.sync.dma_start(out=colrel_sb[:], in_=colrel_d[:])

                # ---- pass 1 ----
                for s in range(NSUP):
                    zT = wp.tile([P, BLK_EDGES], bf16, tag="zT")
                    nc.sync.dma_start(
                        out=zT[:], in_=zt_d[:, s * BLK_EDGES:(s + 1) * BLK_EDGES])
                    ps1 = psp.tile([P, 512], f32, tag="ps1", space="PSUM")
                    nc.tensor.matmul(out=ps1[:], lhsT=w1a2[:, 0:P],
                                     rhs=zT[:, 0:512], start=True, stop=False)
                    nc.tensor.matmul(out=ps1[:], lhsT=w1a2[:, P:2 * P],
                                     rhs=zT[:, 512:1024], start=False, stop=True)
                    nc.scalar.activation(
                        out=h1_store[:, s * 512:(s + 1) * 512], in_=ps1[:],
                        func=AF.Copy)
                    nc.vector.bn_stats(out=bnst1[:, s * 6:(s + 1) * 6],
                                       in_=ps1[:])

                # ---- BN1 stats all-reduce + fold ----
                fold_stats(bnst1, NSUP, NSUP * 512, bn1, 1.0 / E,
                           s1d, t1d, "1")

                # ---- pass 2: scatter ----
                with tc.tile_pool(name="psum2", bufs=2, space="PSUM") as psb:
                    for s in range(NSUP):
                        rT = wp.tile([P, 512], bf16, tag="rT")
                        nc.scalar.activation(
                            out=rT[:], in_=h1_store[:, s * 512:(s + 1) * 512],
                            func=AF.Relu, bias=t1d[:, 0:1], scale=s1d[:, 0:1])
                        ps_tr = psp.tile([P, 512], bf16, tag="ps_tr",
                                         space="PSUM")
                        for j in range(4):
                            nc.tensor.transpose(
                                out=ps_tr[:, j * 128:(j + 1) * 128],
                                in_=rT[:, j * 128:(j + 1) * 128],
                                identity=ident[:])
                        rt = wp.tile([P, 512], bf16, tag="rt")
                        nc.vector.tensor_copy(out=rt[:], in_=ps_tr[:])
                        oh = wp.tile([P, 8 * P], bf16, tag="oh")
                        for k in range(8):
                            nc.vector.tensor_scalar(
                                out=oh[:, k * P:(k + 1) * P], in0=iota_bf[:],
                                scalar1=colrel_sb[:, s * 8 + k:s * 8 + k + 1],
                                scalar2=None, op0=OP.is_equal)
                        ps_blk = psb.tile([P, F], f32, tag="ps_blk",
                                          space="PSUM")
                        for k in range(8):
                            if k < 4:
                                rhs = rt[:, k * 128:k * 128 + F]
                            else:
                                rhs = rt[:, (k - 4) * 128 + F:(k - 3) * 128]
                            nc.tensor.matmul(out=ps_blk[:],
                                             lhsT=oh[:, k * P:(k + 1) * P],
                                             rhs=rhs,
                                             start=(k == 0), stop=(k == 7))
                        nc.vector.tensor_copy(
                            out=sums_sb[:, s * F:(s + 1) * F], in_=ps_blk[:])

            # ---------------- node phase ---------------------------------
            with (
                tc.tile_pool(name="node_persist", bufs=1) as np_,
                tc.tile_pool(name="nwork", bufs=3) as nw,
                tc.tile_pool(name="psum3", bufs=2, space="PSUM") as ps3,
            ):
                h2_store = np_.tile([P, NPAIR * 512], f32)
                bnst2 = np_.tile([P, NPAIR * 6], f32)

                for pr in range(NPAIR):
                    z2 = nw.tile([P, 1024], bf16, tag="z2")
                    nc.sync.dma_start(
                        out=z2[0:F, :],
                        in_=xt_d[:, pr * 1024:(pr + 1) * 1024])
                    ps_h2 = ps3.tile([P, 512], f32, tag="ps_h2", space="PSUM")
                    for g2 in range(2):
                        gidx = pr * 2 + g2
                        me4 = nw.tile([P, 4 * 65], bf16, tag="me4")
                        ps_mT = ps3.tile([P, 512], bf16, tag="ps_mT",
                                         space="PSUM")
                        for j in range(4):
                            b = gidx * 4 + j
                            nc.vector.tensor_scalar(
                                out=me4[:, j * 65:j * 65 + F],
                                in0=sums_sb[:, b * F:(b + 1) * F],
                                scalar1=recip_sb[:, b:b + 1],
                                scalar2=None, op0=OP.mult)
                            nc.vector.tensor_copy(
                                out=me4[:, j * 65 + F:j * 65 + 65],
                                in_=ind_sb[:, b:b + 1])
                            nc.tensor.transpose(
                                out=ps_mT[0:F + 1, j * 128:(j + 1) * 128],
                                in_=me4[:, j * 65:(j + 1) * 65],
                                identity=ident[:])
                        mT = nw.tile([F + 1, 512], bf16, tag="mT")
                        nc.vector.tensor_copy(out=mT[:], in_=ps_mT[0:F + 1, :])
                        ps_msg = ps3.tile([F, 512], f32, tag="ps_msg",
                                          space="PSUM")
                        nc.tensor.matmul(out=ps_msg[:], lhsT=w2a[:], rhs=mT[:],
                                         start=True, stop=True)
                        nc.scalar.activation(
                            out=z2[F:P, g2 * 512:(g2 + 1) * 512],
                            in_=ps_msg[:], func=AF.Copy)
                    nc.tensor.matmul(out=ps_h2[:], lhsT=w1b2[:, 0:P],
                                     rhs=z2[:, 0:512], start=True, stop=False)
                    nc.tensor.matmul(out=ps_h2[:], lhsT=w1b2[:, P:2 * P],
                                     rhs=z2[:, 512:1024], start=False,
                                     stop=True)
                    nc.scalar.activation(
                        out=h2_store[:, pr * 512:(pr + 1) * 512],
                        in_=ps_h2[:], func=AF.Copy)
                    nc.vector.bn_stats(out=bnst2[:, pr * 6:(pr + 1) * 6],
                                       in_=ps_h2[:])

                # ---- BN2 stats all-reduce + fold ----
                fold_stats(bnst2, NPAIR, NPAIR * 512, bn2, 1.0 / N,
                           s2d, t2d, "2")

                # ---- output ----
                for pr in range(NPAIR):
                    rT2 = nw.tile([P, 512], bf16, tag="rT2")
                    nc.scalar.activation(
                        out=rT2[:], in_=h2_store[:, pr * 512:(pr + 1) * 512],
                        func=AF.Relu, bias=t2d[:, 0:1], scale=s2d[:, 0:1])
                    ps_o = ps3.tile([P, 512], f32, tag="ps_o", space="PSUM")
                    nc.tensor.matmul(out=ps_o[:], lhsT=w2bd[:], rhs=rT2[:],
                                     start=True, stop=True)
                    oT = nw.tile([P, 512], f32, tag="oT")
                    nc.scalar.activation(out=oT[:], in_=ps_o[:],
                                         func=AF.Identity, bias=b2b[:, 0:1],
                                         scale=1.0)
                    nc.sync.dma_start(
                        out=out_d[:, pr * 512:(pr + 1) * 512], in_=oT[:])

    nc.compile()
    return nc


# --------------------------------------------------------------------------
# Entry point
# --------------------------------------------------------------------------

def _weights_map(W1a, b1a, g1, be1, W2a, b2a, W1b, b1b, g2, be2, W2b, b2b):
    import ml_dtypes
    bf16 = ml_dtypes.bfloat16
    W1a = np.asarray(W1a, np.float32)
    W1b = np.asarray(W1b, np.float32)
    W2a = np.asarray(W2a, np.float32)
    W2b = np.asarray(W2b, np.float32)

    w1a2 = np.zeros((DIN, 2 * P), np.float32)
    w1a2[:, 0:F] = W1a
    w1a2[:, P + F:2 * P] = W1a
    w1b2 = np.zeros((DIN, 2 * P), np.float32)
    w1b2[:, 0:F] = W1b
    w1b2[:, P + F:2 * P] = W1b
    w2bd = np.zeros((DIN, P), np.float32)
    w2bd[0:F, 0:F] = W2b
    w2bd[F:P, F:P] = W2b
    w2a_ext = np.concatenate(
        [W2a, np.asarray(b2a, np.float32)[None, :]], axis=0)
    bn1 = np.stack([np.asarray(g1, np.float32),
                    np.asarray(be1, np.float32)], axis=1)
    bn2 = np.stack([np.asarray(g2, np.float32),
                    np.asarray(be2, np.float32)], axis=1)
    b2b_dup = np.concatenate([np.asarray(b2b, np.float32)] * 2)[:, None]
    return {
        "w1a2": np.ascontiguousarray(w1a2.astype(bf16)),
        "w2a_ext": np.ascontiguousarray(w2a_ext.astype(bf16)),
        "w1b2": np.ascontiguousarray(w1b2.astype(bf16)),
        "w2b_bd": np.ascontiguousarray(w2bd.astype(bf16)),
        "bn1": np.ascontiguousarray(bn1),
        "bn2": np.ascontiguousarray(bn2),
        "b2b_dup": np.ascontiguousarray(b2b_dup),
    }


def _prepare(inputs):
    in_maps, perms, nsup = _shard_inputs(
        inputs["x"], inputs["edge_index"], inputs["edge_attr"])
    if nsup not in _BUILD_CACHE:
        _BUILD_CACHE[nsup] = _build_program(nsup)
    nc = _BUILD_CACHE[nsup]
    wmap = _weights_map(
        inputs["W1a"], inputs["b1a"], inputs["g1"], inputs["be1"],
        inputs["W2a"], inputs["b2a"], inputs["W1b"], inputs["b1b"],
        inputs["g2"], inputs["be2"], inputs["W2b"], inputs["b2b"])
    for m in in_maps:
        m.update(wmap)
    return nc, in_maps, perms, nsup


def _unshard(results, perms, nsup):
    out = np.empty((N, F), dtype=np.float32)
    npair = nsup // 8
    for c in range(NCORES):
        oT = results[c]["outT"]                     # [128, NPAIR*512]
        o3 = oT.reshape(P, npair, 512)
        full = np.empty((nsup * P, F), dtype=np.float32)
        fullg = full.reshape(npair, 2, 512, F)
        fullg[:, 0] = o3[0:F].transpose(1, 2, 0)
        fullg[:, 1] = o3[F:P].transpose(1, 2, 0)
        perm = perms[c]
        valid = perm >= 0
        out[c * NPC + perm[valid]] = full[valid]
    return out


def kernel(x, edge_index, edge_attr, u, batch,
           W1a, b1a, g1, be1, W2a, b2a,
           W1b, b1b, g2, be2, W2b, b2b, **_unused):
    from concourse.bass_utils import run_bass_kernel_spmd

    inputs = dict(x=x, edge_index=edge_index, edge_attr=edge_attr,
                  W1a=W1a, b1a=b1a, g1=g1, be1=be1, W2a=W2a, b2a=b2a,
                  W1b=W1b, b1b=b1b, g2=g2, be2=be2, W2b=W2b, b2b=b2b)
    nc, in_maps, perms, nsup = _prepare(inputs)
    res = run_bass_kernel_spmd(nc, in_maps, core_ids=list(range(NCORES)))
    return _unshard(res.results, perms, nsup)
